# revision 33
# baseline (speedup 1.0000x reference)
"""Multi-head causal self-attention (B=2, S=4096, D=512, H=8) on 8 trn2 cores.

Sharding: batch*heads = 16 (b,h) pairs -> 2 heads per core (head-parallel,
qkv weight columns sharded per head group). Zero cross-core communication.

Per-core kernel (heads h0=2g, h1=2g+1 stacked on partition halves):
  - inputs: xt = X[b].T  (512, 4096),  w = [Wq|Wk|Wv] head cols (512, 384)
  - QT/KT: (128, 4096) with partitions 0-63 = head0 dims, 64-127 = head1
  - V: natural layout per 128-row j-tile, with an appended ones column so
    the AV matmul also produces the softmax denominator (row 64 of out).
  - scores computed transposed (keys on partitions) so softmax sum comes
    from the ones column; exp on ScalarE with scale=1/8 folded in; causal
    mask via 4 static mask tiles (DVE multiply) + range restriction.
  - AV accumulated in PSUM over j-tiles; O.T (65, 512) per (head, range)
    is copied to SBUF and DMA'd out UNNORMALIZED and TRANSPOSED; the host
    divides by the denominator row and transposes (O(S*D) epilogue).
  - ScalarE (exp) is the bottleneck engine (~1 col/cycle @1.2GHz, ~112us
    of streaming); the pair loop emits scores(p+1) BEFORE av(p) so the
    PE never head-of-line-blocks the score feed while waiting on exp.
All matmuls bf16 (1 cycle/row). QK/V projections are emitted just-in-time
inside the attention loop on a dedicated PSUM aux bank so the score
double-buffer rotation is never disturbed.
"""

import os
import sys

import numpy as np

for _p in ("/opt/trn_rl_repo", "/root/.axon_site/_ro/trn_rl_repo"):
    if os.path.isdir(_p) and _p not in sys.path:
        sys.path.append(_p)

import concourse.bass as bass
import concourse.tile as tile
from concourse import mybir

F32 = mybir.dt.float32
BF16 = mybir.dt.bfloat16

B, S, D, H = 2, 4096, 512, 8
HD = 64          # head dim
NHC = 2          # heads per core
P = 128          # partitions
KC = D // P      # 4 contraction chunks for the projection
IT = 512         # query-range width
NI = S // IT     # 8 query ranges
JT = 128         # key-tile width
NJ = S // JT     # 32 key tiles
SCALE = 1.0 / np.sqrt(HD)  # 0.125


def build_nc():
    nc = bass.Bass()
    xt = nc.declare_dram_parameter("xt", [D, S], BF16, isOutput=False)
    w = nc.declare_dram_parameter("w", [D, 3 * P], BF16, isOutput=False)
    # unnormalized transposed output per (head, range): 64 dims + denom row
    ot = nc.declare_dram_parameter("ot", [NHC, NI, 65, IT], F32, isOutput=True)

    with tile.TileContext(nc) as tc:
        with (
            tc.tile_pool(name="singles", bufs=1) as singles,
            tc.tile_pool(name="epool", bufs=6) as epool,
            tc.tile_pool(name="otpool", bufs=4) as otpool,
            tc.tile_pool(name="ps_sc", bufs=2, space="PSUM") as ps_sc,
            tc.tile_pool(name="ps_av", bufs=2, space="PSUM") as ps_av,
            tc.tile_pool(name="ps_aux", bufs=2, space="PSUM") as ps_aux,
        ):
            # ---- resident tensors -------------------------------------
            xt_sb = singles.tile([P, KC, S], BF16, name="xt_sb")
            w_sb = singles.tile([P, KC, 3 * P], BF16, name="w_sb")
            qt = singles.tile([P, S], BF16, name="qt")
            kt = singles.tile([P, S], BF16, name="kt")
            # V per j-tile: [jt, 0:64] head0, [jt, 64] ones, [jt, 65:129]
            # head1, [jt, 129] ones
            v_sb = singles.tile([P, NJ, 130], BF16, name="v_sb")
            masks_f = singles.tile([P, 4, IT], F32, name="masks_f")
            masks = singles.tile([P, 4, IT], BF16, name="masks")
            zbias = singles.tile([P, 1], F32, name="zbias")
            dummy_e = singles.tile([P, 2], BF16, name="dummy_e")

            # ---- loads ------------------------------------------------
            # w first, then xt sliced so the col-0:512 chunks (all 4 c)
            # land first -> QK(0) + V(0..3) can start ~3us after issue.
            w_r = w[:, :].rearrange("(c p) n -> p c n", p=P)
            nc.sync.dma_start(out=w_sb[:, :, 0 : 2 * P], in_=w_r[:, :, 0 : 2 * P])
            xt_r = xt[:, :].rearrange("(c p) s -> p c s", p=P)
            for c in range(KC):
                nc.sync.dma_start(out=xt_sb[:, c, 0:512], in_=xt_r[:, c, 0:512])
            nc.sync.dma_start(
                out=w_sb[:, :, 2 * P : 3 * P], in_=w_r[:, :, 2 * P : 3 * P]
            )
            for lo, hi in ((512, 2048), (2048, 4096)):
                for c in range(KC):
                    nc.sync.dma_start(
                        out=xt_sb[:, c, lo:hi], in_=xt_r[:, c, lo:hi]
                    )

            # ---- constants (no DMA deps; overlap the loads) -----------
            nc.vector.memset(zbias, 0.0)
            # ones columns for the denominator rows
            nc.vector.memset(v_sb[:, :, 64:65], 1.0)
            nc.vector.memset(v_sb[:, :, 129:130], 1.0)
            # mask k: keep (=1) iff x - p - 128k >= 0, else 0
            for k in range(4):
                nc.gpsimd.memset(masks_f[:, k, :], 1.0)
                nc.gpsimd.affine_select(
                    out=masks_f[:, k, :],
                    in_=masks_f[:, k, :],
                    compare_op=mybir.AluOpType.is_ge,
                    fill=0.0,
                    base=-JT * k,
                    pattern=[[1, IT]],
                    channel_multiplier=-1,
                )
            nc.vector.tensor_copy(masks, masks_f)

            exp_f = mybir.ActivationFunctionType.Exp
            # preload the exp table set during the DMA wait
            nc.scalar.activation(
                dummy_e, masks_f[:, 0, 0:2], exp_f, bias=zbias, scale=SCALE
            )
            # PE heater: fp32 matmuls (4 cyc/row) with no DMA deps warm the
            # PE p-state (0.65 -> 2.4 GHz ramp) while the first xt slices
            # are still in flight.
            hp = ps_aux.tile([P, IT], F32, tag="aux", name="heat")
            for _ in range(4):
                nc.tensor.matmul(
                    hp[:, 0:P],
                    lhsT=masks_f[:, 0, 0:P],
                    rhs=masks_f[:, 1, 0:P],
                    start=True,
                    stop=True,
                )

            # ---- projections (JIT, on the aux bank) -------------------
            def project_v(j):
                ps_v = ps_aux.tile([P, IT], F32, tag="aux", name="ps_v")
                for c in range(KC):
                    nc.tensor.matmul(
                        ps_v[:, 0:P],
                        lhsT=xt_sb[:, c, j * JT : (j + 1) * JT],
                        rhs=w_sb[:, c, 2 * P : 3 * P],
                        start=(c == 0),
                        stop=(c == KC - 1),
                    )
                nc.vector.tensor_copy(v_sb[:, j, 0:64], ps_v[:, 0:64])
                nc.vector.tensor_copy(v_sb[:, j, 65:129], ps_v[:, 64:128])

            def project_qk_half(r, half):
                # half 0 -> Q, half 1 -> K; out = w_half.T @ xt_chunk
                sl = slice(r * IT, (r + 1) * IT)
                ps_q = ps_aux.tile([P, IT], F32, tag="aux", name="ps_q")
                for c in range(KC):
                    nc.tensor.matmul(
                        ps_q,
                        lhsT=w_sb[:, c, half * P : (half + 1) * P],
                        rhs=xt_sb[:, c, sl],
                        start=(c == 0),
                        stop=(c == KC - 1),
                    )
                dst = qt if half == 0 else kt
                nc.vector.tensor_copy(dst[:, sl], ps_q)

            # head: QK(0) first (unblocks the exp stream), then V(0..11)
            # (ranges 0-2); V(12..31) are projected inside the loop.
            project_qk_half(0, 0)
            project_qk_half(0, 1)

            # ---- attention --------------------------------------------
            # software pipelined: per pair p, ACT exps pair p while the PE
            # emits scores(p+1) FIRST (keeps ACT fed), then fill work
            # (projections), then av(p) which waits on exp(p).
            def pair_meta(p_i, njt):
                offs = []
                for u in (0, 1):
                    k = 2 * p_i + u - (njt - 4)
                    offs.append(JT * k if k > 0 else 0)
                return offs, 2 * p_i >= njt - 4

            def emit_scores(p_i, i0, njt):
                offs, diag = pair_meta(p_i, njt)
                sc = [
                    ps_sc.tile([P, 2 * IT], F32, tag="sc", name=f"sc{h}")
                    for h in range(NHC)
                ]
                e = [
                    epool.tile([P, 2 * IT], BF16, tag="e", name=f"e{h}")
                    for h in range(NHC)
                ]
                for u in (0, 1):
                    j = 2 * p_i + u
                    for h in range(NHC):
                        hsl = slice(64 * h, 64 * (h + 1))
                        nc.tensor.matmul(
                            sc[h][:, u * IT + offs[u] : (u + 1) * IT],
                            lhsT=kt[hsl, j * JT : (j + 1) * JT],
                            rhs=qt[hsl, i0 + offs[u] : i0 + IT],
                            start=True,
                            stop=True,
                            tile_position=(64 * h, 0),
                        )
                return sc, e, offs, diag

            def emit_exps(state, p_i, njt):
                sc, e, offs, diag = state
                for h in range(NHC):
                    if not diag:
                        nc.scalar.activation(
                            e[h], sc[h], exp_f, bias=zbias, scale=SCALE
                        )
                    else:
                        for u in (0, 1):
                            k = 2 * p_i + u - (njt - 4)
                            usl = slice(u * IT + offs[u], (u + 1) * IT)
                            nc.scalar.activation(
                                e[h][:, usl], sc[h][:, usl], exp_f,
                                bias=zbias, scale=SCALE,
                            )
                            if k >= 0:
                                nc.vector.tensor_mul(
                                    e[h][:, usl], e[h][:, usl],
                                    masks[:, k, offs[u] : IT],
                                )

            def emit_avs(state, av, p_i, njt):
                _, e, offs, _ = state
                for h in range(NHC):
                    for u in (0, 1):
                        j = 2 * p_i + u
                        nc.tensor.matmul(
                            av[h][:, offs[u] : IT],
                            lhsT=v_sb[:, j, 65 * h : 65 * h + 65],
                            rhs=e[h][:, u * IT + offs[u] : (u + 1) * IT],
                            start=(j == 0),
                            stop=(j == njt - 1),
                        )

            # first scores ASAP (feeds ACT); V(0..3) right after (range 0
            # AV needs them only ~1us later); V(4..31) JIT inside the loop.
            state = emit_scores(0, 0, 4)
            for j in range(4):
                project_v(j)

            def heat(width):
                # dummy matmul into the aux bank: denies the HAM clock gate
                # a PE-idle window during ACT-bound stretches. Single weight
                # column -> 1/128 of the array is active: busy-time without
                # the chip-level power draw (all 8 cores run this kernel).
                hp2 = ps_aux.tile([P, IT], F32, tag="aux", name="heat2")
                nc.tensor.matmul(
                    hp2[0:1, 0:width],
                    lhsT=masks[:, 2, 0:1],
                    rhs=masks[:, 3, 0:width],
                    start=True,
                    stop=True,
                )

            for t in range(NI):
                i0 = t * IT
                njt = 4 * (t + 1)         # causal: j-tiles 0..njt-1
                npairs = njt // 2
                av = [
                    ps_av.tile([65, IT], F32, tag="av", name=f"av{h}")
                    for h in range(NHC)
                ]
                # V j-tiles for range t+1, spread across pairs (lag-1)
                vjs = list(range(4 * t + 4, 4 * t + 8)) if t + 1 < NI else []
                for p_i in range(npairs):
                    emit_exps(state, p_i, njt)
                    prev = state
                    # Q/K projection for the next range BEFORE the range
                    # boundary scores that read them
                    if t + 1 < NI:
                        if p_i == 0:
                            project_qk_half(t + 1, 0)
                        elif p_i == 1:
                            project_qk_half(t + 1, 1)
                    if p_i + 1 < npairs:
                        state = emit_scores(p_i + 1, i0, njt)
                    elif t + 1 < NI:
                        state = emit_scores(0, i0 + IT, 4 * (t + 2))
                    else:
                        state = None
                    if npairs <= 4:
                        mine = vjs[2 * p_i : 2 * p_i + 2]
                    else:
                        mine = vjs[p_i - 2 : p_i - 1] if p_i >= 2 else []
                    for j in mine:
                        project_v(j)
                    if (
                        t >= 1 and not mine
                        and (p_i >= 2 or t + 1 == NI)
                        and not (t + 1 == NI and p_i >= npairs - 2)
                    ):
                        heat(384)
                    emit_avs(prev, av, p_i, njt)
                # copy O.T out of PSUM (frees the accumulators) and DMA the
                # unnormalized transposed block; host divides + transposes
                for h in range(NHC):
                    o = otpool.tile([65, IT], F32, tag="ot", name=f"ot{h}")
                    nc.vector.tensor_copy(o, av[h])
                    nc.sync.dma_start(out=ot[h, t], in_=o)
    return nc


def legalize_waits(nc):
    """This toolchain's walrus allows at most ONE sync-wait per instruction;
    split extra waits onto preceding same-engine NoOps (same trick Tile uses
    for its own wait/update carriers)."""
    nsplit = 0
    for f in nc.m.functions:
        for blk in f.blocks:
            new_insts = []
            for inst in blk.instructions:
                si = getattr(inst, "sync_info", None)
                ow = list(si.on_wait) if (si is not None and si.on_wait) else []
                if len(ow) > 1:
                    for w_i, wcond in enumerate(ow[:-1]):
                        nsplit += 1
                        nop = mybir.InstNoOp(
                            name=f"{inst.name}-wsplit{w_i}",
                            sync_info=mybir.SyncInfo(on_wait=[wcond], on_update=[]),
                            bass_nofuse=True,
                            engine=inst.engine,
                        )
                        new_insts.append(nop)
                    si.on_wait = ow[-1:]
                new_insts.append(inst)
            try:
                blk.instructions[:] = new_insts
            except TypeError:
                blk.instructions = new_insts
    return nsplit


_NC_CACHE = None


def _get_nc():
    global _NC_CACHE
    if _NC_CACHE is None:
        nc = build_nc()
        legalize_waits(nc)
        _NC_CACHE = nc
    return _NC_CACHE


def shard_inputs(inputs, qkv_weights):
    import ml_dtypes

    bf16 = ml_dtypes.bfloat16
    x = np.ascontiguousarray(np.asarray(inputs, dtype=np.float32))
    wf = np.ascontiguousarray(np.asarray(qkv_weights, dtype=np.float32))
    in_maps = []
    for c in range(8):
        b, g = divmod(c, 4)
        lo = g * P
        xt_c = np.ascontiguousarray(x[b].T).astype(bf16)
        w_c = np.ascontiguousarray(
            np.concatenate(
                [wf[:, q * D + lo : q * D + lo + P] for q in range(3)], axis=1
            )
        ).astype(bf16)
        in_maps.append({"xt": xt_c, "w": w_c})
    return in_maps


def gather_outputs(results):
    out = np.empty((B, S, D), dtype=np.float32)
    for c in range(8):
        b, g = divmod(c, 4)
        ot = results[c]["ot"]  # [NHC, NI, 65, IT]
        for h in range(NHC):
            blk = ot[h].transpose(1, 0, 2).reshape(65, S)  # [65, S]
            out[b, :, g * P + h * HD : g * P + (h + 1) * HD] = (
                blk[0:64] / blk[64:65]
            ).T
    return out


def run(in_maps, **kwargs):
    from concourse.bass_utils import run_bass_kernel_spmd

    return run_bass_kernel_spmd(_get_nc(), in_maps, list(range(8)), **kwargs)


def kernel(**inputs):
    in_maps = shard_inputs(inputs["inputs"], inputs["qkv_weights"])
    res = run(in_maps)
    return gather_outputs(res.results)


# revision 34
# speedup vs baseline: 1.0116x; 1.0116x over previous
"""Multi-head causal self-attention (B=2, S=4096, D=512, H=8) on 8 trn2 cores.

Sharding: batch*heads = 16 (b,h) pairs -> 2 heads per core (head-parallel,
qkv weight columns sharded per head group). Zero cross-core communication.

Per-core kernel (heads h0=2g, h1=2g+1 stacked on partition halves):
  - inputs: xt = X[b].T  (512, 4096),  w = [Wq|Wk|Wv] head cols (512, 384)
  - QT/KT: (128, 4096) with partitions 0-63 = head0 dims, 64-127 = head1
  - V: natural layout per 128-row j-tile, with an appended ones column so
    the AV matmul also produces the softmax denominator (row 64 of out).
  - scores computed transposed (keys on partitions) so softmax sum comes
    from the ones column; exp on ScalarE with scale=1/8 folded in; causal
    mask via 4 static mask tiles (DVE multiply) + range restriction.
  - AV accumulated in PSUM over j-tiles; O.T (65, 512) per (head, range)
    is copied to SBUF and DMA'd out UNNORMALIZED and TRANSPOSED; the host
    divides by the denominator row and transposes (O(S*D) epilogue).
  - ScalarE (exp) is the bottleneck engine (~1 col/cycle @1.2GHz, ~112us
    of streaming); the pair loop emits scores(p+1) BEFORE av(p) so the
    PE never head-of-line-blocks the score feed while waiting on exp.
All matmuls bf16 (1 cycle/row). QK/V projections are emitted just-in-time
inside the attention loop on a dedicated PSUM aux bank so the score
double-buffer rotation is never disturbed.
"""

import os
import sys

import numpy as np

for _p in ("/opt/trn_rl_repo", "/root/.axon_site/_ro/trn_rl_repo"):
    if os.path.isdir(_p) and _p not in sys.path:
        sys.path.append(_p)

import concourse.bass as bass
import concourse.tile as tile
from concourse import mybir

F32 = mybir.dt.float32
BF16 = mybir.dt.bfloat16

B, S, D, H = 2, 4096, 512, 8
HD = 64          # head dim
NHC = 2          # heads per core
P = 128          # partitions
KC = D // P      # 4 contraction chunks for the projection
IT = 512         # query-range width
NI = S // IT     # 8 query ranges
JT = 128         # key-tile width
NJ = S // JT     # 32 key tiles
SCALE = 1.0 / np.sqrt(HD)  # 0.125


def build_nc():
    nc = bass.Bass()
    xt = nc.declare_dram_parameter("xt", [D, S], BF16, isOutput=False)
    w = nc.declare_dram_parameter("w", [D, 3 * P], BF16, isOutput=False)
    # unnormalized transposed output per (head, range): 64 dims + denom row
    ot = nc.declare_dram_parameter("ot", [NHC, NI, 65, IT], F32, isOutput=True)

    with tile.TileContext(nc) as tc:
        with (
            tc.tile_pool(name="singles", bufs=1) as singles,
            tc.tile_pool(name="epool", bufs=6) as epool,
            tc.tile_pool(name="otpool", bufs=4) as otpool,
            tc.tile_pool(name="ps_sc", bufs=2, space="PSUM") as ps_sc,
            tc.tile_pool(name="ps_av", bufs=2, space="PSUM") as ps_av,
            tc.tile_pool(name="ps_aux", bufs=2, space="PSUM") as ps_aux,
        ):
            # ---- resident tensors -------------------------------------
            xt_sb = singles.tile([P, KC, S], BF16, name="xt_sb")
            w_sb = singles.tile([P, KC, 3 * P], BF16, name="w_sb")
            qt = singles.tile([P, S], BF16, name="qt")
            kt = singles.tile([P, S], BF16, name="kt")
            # V per j-tile: [jt, 0:64] head0, [jt, 64] ones, [jt, 65:129]
            # head1, [jt, 129] ones
            v_sb = singles.tile([P, NJ, 130], BF16, name="v_sb")
            masks_f = singles.tile([P, 4, IT], F32, name="masks_f")
            masks = singles.tile([P, 4, IT], BF16, name="masks")
            zbias = singles.tile([P, 1], F32, name="zbias")
            dummy_e = singles.tile([P, 2], BF16, name="dummy_e")

            # ---- loads ------------------------------------------------
            # w first, then xt sliced so the col-0:512 chunks (all 4 c)
            # land first -> QK(0) + V(0..3) can start ~3us after issue.
            w_r = w[:, :].rearrange("(c p) n -> p c n", p=P)
            nc.sync.dma_start(out=w_sb[:, :, 0 : 2 * P], in_=w_r[:, :, 0 : 2 * P])
            xt_r = xt[:, :].rearrange("(c p) s -> p c s", p=P)
            for c in range(KC):
                nc.sync.dma_start(out=xt_sb[:, c, 0:512], in_=xt_r[:, c, 0:512])
            nc.sync.dma_start(
                out=w_sb[:, :, 2 * P : 3 * P], in_=w_r[:, :, 2 * P : 3 * P]
            )
            for lo, hi in ((512, 2048), (2048, 4096)):
                for c in range(KC):
                    nc.sync.dma_start(
                        out=xt_sb[:, c, lo:hi], in_=xt_r[:, c, lo:hi]
                    )

            # ---- constants (no DMA deps; overlap the loads) -----------
            nc.vector.memset(zbias, 0.0)
            # ones columns for the denominator rows
            nc.vector.memset(v_sb[:, :, 64:65], 1.0)
            nc.vector.memset(v_sb[:, :, 129:130], 1.0)
            # mask k: keep (=1) iff x - p - 128k >= 0, else 0
            for k in range(4):
                nc.gpsimd.memset(masks_f[:, k, :], 1.0)
                nc.gpsimd.affine_select(
                    out=masks_f[:, k, :],
                    in_=masks_f[:, k, :],
                    compare_op=mybir.AluOpType.is_ge,
                    fill=0.0,
                    base=-JT * k,
                    pattern=[[1, IT]],
                    channel_multiplier=-1,
                )
            nc.vector.tensor_copy(masks, masks_f)

            exp_f = mybir.ActivationFunctionType.Exp
            # preload the exp table set during the DMA wait
            nc.scalar.activation(
                dummy_e, masks_f[:, 0, 0:2], exp_f, bias=zbias, scale=SCALE
            )
            # PE heater: fp32 matmuls (4 cyc/row) with no DMA deps warm the
            # PE p-state (0.65 -> 2.4 GHz ramp) while the first xt slices
            # are still in flight.
            hp = ps_aux.tile([P, IT], F32, tag="aux", name="heat")
            for _ in range(4):
                nc.tensor.matmul(
                    hp[:, 0:P],
                    lhsT=masks_f[:, 0, 0:P],
                    rhs=masks_f[:, 1, 0:P],
                    start=True,
                    stop=True,
                )

            # ---- projections (JIT, on the aux bank) -------------------
            def project_v(j):
                ps_v = ps_aux.tile([P, IT], F32, tag="aux", name="ps_v")
                for c in range(KC):
                    nc.tensor.matmul(
                        ps_v[:, 0:P],
                        lhsT=xt_sb[:, c, j * JT : (j + 1) * JT],
                        rhs=w_sb[:, c, 2 * P : 3 * P],
                        start=(c == 0),
                        stop=(c == KC - 1),
                    )
                nc.vector.tensor_copy(v_sb[:, j, 0:64], ps_v[:, 0:64])
                nc.vector.tensor_copy(v_sb[:, j, 65:129], ps_v[:, 64:128])

            def project_qk_half(r, half):
                # half 0 -> Q, half 1 -> K; out = w_half.T @ xt_chunk
                sl = slice(r * IT, (r + 1) * IT)
                ps_q = ps_aux.tile([P, IT], F32, tag="aux", name="ps_q")
                for c in range(KC):
                    nc.tensor.matmul(
                        ps_q,
                        lhsT=w_sb[:, c, half * P : (half + 1) * P],
                        rhs=xt_sb[:, c, sl],
                        start=(c == 0),
                        stop=(c == KC - 1),
                    )
                dst = qt if half == 0 else kt
                nc.vector.tensor_copy(dst[:, sl], ps_q)

            # head: QK(0) first (unblocks the exp stream), then V(0..11)
            # (ranges 0-2); V(12..31) are projected inside the loop.
            project_qk_half(0, 0)
            project_qk_half(0, 1)

            # ---- attention --------------------------------------------
            # software pipelined: per pair p, ACT exps pair p while the PE
            # emits scores(p+1) FIRST (keeps ACT fed), then fill work
            # (projections), then av(p) which waits on exp(p).
            def pair_meta(p_i, njt):
                offs = []
                for u in (0, 1):
                    k = 2 * p_i + u - (njt - 4)
                    offs.append(JT * k if k > 0 else 0)
                return offs, 2 * p_i >= njt - 4

            def emit_scores(p_i, i0, njt):
                offs, diag = pair_meta(p_i, njt)
                sc = [
                    ps_sc.tile([P, 2 * IT], F32, tag="sc", name=f"sc{h}")
                    for h in range(NHC)
                ]
                e = [
                    epool.tile([P, 2 * IT], BF16, tag="e", name=f"e{h}")
                    for h in range(NHC)
                ]
                for u in (0, 1):
                    j = 2 * p_i + u
                    for h in range(NHC):
                        hsl = slice(64 * h, 64 * (h + 1))
                        nc.tensor.matmul(
                            sc[h][:, u * IT + offs[u] : (u + 1) * IT],
                            lhsT=kt[hsl, j * JT : (j + 1) * JT],
                            rhs=qt[hsl, i0 + offs[u] : i0 + IT],
                            start=True,
                            stop=True,
                            tile_position=(64 * h, 0),
                        )
                return sc, e, offs, diag

            def emit_exps(state, p_i, njt):
                sc, e, offs, diag = state
                for h in range(NHC):
                    if not diag:
                        nc.scalar.activation(
                            e[h], sc[h], exp_f, bias=zbias, scale=SCALE
                        )
                    else:
                        for u in (0, 1):
                            k = 2 * p_i + u - (njt - 4)
                            usl = slice(u * IT + offs[u], (u + 1) * IT)
                            nc.scalar.activation(
                                e[h][:, usl], sc[h][:, usl], exp_f,
                                bias=zbias, scale=SCALE,
                            )
                            if k >= 0:
                                nc.vector.tensor_mul(
                                    e[h][:, usl], e[h][:, usl],
                                    masks[:, k, offs[u] : IT],
                                )

            def emit_avs(state, av, p_i, njt):
                _, e, offs, _ = state
                for h in range(NHC):
                    for u in (0, 1):
                        j = 2 * p_i + u
                        nc.tensor.matmul(
                            av[h][:, offs[u] : IT],
                            lhsT=v_sb[:, j, 65 * h : 65 * h + 65],
                            rhs=e[h][:, u * IT + offs[u] : (u + 1) * IT],
                            start=(j == 0),
                            stop=(j == njt - 1),
                        )

            # first scores ASAP (feeds ACT); V(0..3) right after (range 0
            # AV needs them only ~1us later); V(4..31) JIT inside the loop.
            state = emit_scores(0, 0, 4)
            for j in range(4):
                project_v(j)

            def heat(width):
                # dummy matmul into the aux bank: denies the HAM clock gate
                # a PE-idle window during ACT-bound stretches
                hp2 = ps_aux.tile([P, IT], F32, tag="aux", name="heat2")
                nc.tensor.matmul(
                    hp2[:, 0:width],
                    lhsT=masks[:, 2, 0:P],
                    rhs=masks[:, 3, 0:width],
                    start=True,
                    stop=True,
                )

            for t in range(NI):
                i0 = t * IT
                njt = 4 * (t + 1)         # causal: j-tiles 0..njt-1
                npairs = njt // 2
                av = [
                    ps_av.tile([65, IT], F32, tag="av", name=f"av{h}")
                    for h in range(NHC)
                ]
                # V j-tiles for range t+1, spread across pairs (lag-1)
                vjs = list(range(4 * t + 4, 4 * t + 8)) if t + 1 < NI else []
                for p_i in range(npairs):
                    emit_exps(state, p_i, njt)
                    prev = state
                    # Q/K projection for the next range BEFORE the range
                    # boundary scores that read them
                    if t + 1 < NI:
                        if p_i == 0:
                            project_qk_half(t + 1, 0)
                        elif p_i == 1:
                            project_qk_half(t + 1, 1)
                    if p_i + 1 < npairs:
                        state = emit_scores(p_i + 1, i0, njt)
                    elif t + 1 < NI:
                        state = emit_scores(0, i0 + IT, 4 * (t + 2))
                    else:
                        state = None
                    if npairs <= 4:
                        mine = vjs[2 * p_i : 2 * p_i + 2]
                    else:
                        mine = vjs[p_i - 2 : p_i - 1] if p_i >= 2 else []
                    for j in mine:
                        project_v(j)
                    if (
                        t >= 1 and not mine
                        and (p_i >= 2 or t + 1 == NI)
                        and not (t + 1 == NI and p_i >= npairs - 2)
                    ):
                        heat(384)
                    emit_avs(prev, av, p_i, njt)
                # copy O.T out of PSUM (frees the accumulators) and DMA the
                # unnormalized transposed block; host divides + transposes
                for h in range(NHC):
                    o = otpool.tile([65, IT], F32, tag="ot", name=f"ot{h}")
                    nc.vector.tensor_copy(o, av[h])
                    nc.sync.dma_start(out=ot[h, t], in_=o)
    return nc


def legalize_waits(nc):
    """This toolchain's walrus allows at most ONE sync-wait per instruction;
    split extra waits onto preceding same-engine NoOps (same trick Tile uses
    for its own wait/update carriers)."""
    nsplit = 0
    for f in nc.m.functions:
        for blk in f.blocks:
            new_insts = []
            for inst in blk.instructions:
                si = getattr(inst, "sync_info", None)
                ow = list(si.on_wait) if (si is not None and si.on_wait) else []
                if len(ow) > 1:
                    for w_i, wcond in enumerate(ow[:-1]):
                        nsplit += 1
                        nop = mybir.InstNoOp(
                            name=f"{inst.name}-wsplit{w_i}",
                            sync_info=mybir.SyncInfo(on_wait=[wcond], on_update=[]),
                            bass_nofuse=True,
                            engine=inst.engine,
                        )
                        new_insts.append(nop)
                    si.on_wait = ow[-1:]
                new_insts.append(inst)
            try:
                blk.instructions[:] = new_insts
            except TypeError:
                blk.instructions = new_insts
    return nsplit


_NC_CACHE = None


def _get_nc():
    global _NC_CACHE
    if _NC_CACHE is None:
        nc = build_nc()
        legalize_waits(nc)
        _NC_CACHE = nc
    return _NC_CACHE


def shard_inputs(inputs, qkv_weights):
    import ml_dtypes

    bf16 = ml_dtypes.bfloat16
    x = np.ascontiguousarray(np.asarray(inputs, dtype=np.float32))
    wf = np.ascontiguousarray(np.asarray(qkv_weights, dtype=np.float32))
    in_maps = []
    for c in range(8):
        b, g = divmod(c, 4)
        lo = g * P
        xt_c = np.ascontiguousarray(x[b].T).astype(bf16)
        w_c = np.ascontiguousarray(
            np.concatenate(
                [wf[:, q * D + lo : q * D + lo + P] for q in range(3)], axis=1
            )
        ).astype(bf16)
        in_maps.append({"xt": xt_c, "w": w_c})
    return in_maps


def gather_outputs(results):
    out = np.empty((B, S, D), dtype=np.float32)
    for c in range(8):
        b, g = divmod(c, 4)
        ot = results[c]["ot"]  # [NHC, NI, 65, IT]
        for h in range(NHC):
            blk = ot[h].transpose(1, 0, 2).reshape(65, S)  # [65, S]
            out[b, :, g * P + h * HD : g * P + (h + 1) * HD] = (
                blk[0:64] / blk[64:65]
            ).T
    return out


def run(in_maps, **kwargs):
    from concourse.bass_utils import run_bass_kernel_spmd

    return run_bass_kernel_spmd(_get_nc(), in_maps, list(range(8)), **kwargs)


def kernel(**inputs):
    in_maps = shard_inputs(inputs["inputs"], inputs["qkv_weights"])
    res = run(in_maps)
    return gather_outputs(res.results)
